# revision 1
# baseline (speedup 1.0000x reference)
"""Triangular GEMM C = triu(A)@triu(B), N=4096 fp32, 8 trn2 cores, T=128.

NB=32 block triangle, 5984 units (i<=k<=j), 748/core -- 9.6% over the
elementwise-triangular FLOP floor (vs 19.5% at T=256, 40% at T=512).

Recursive ladder decomposition; single uniform SPMD program (126 chains);
per-core behavior via host packing only. A-cores (0-3) own the triu[0,16)
A-panel as 'tri'; B-cores are the transpose image with A/B roles swapped
and reversed panel packing (suffix reference patterns align).

Per unit: ONE matmul [128c x 128m x 128f] into a [128,128] psum tile.
"""

import numpy as np

N = 4096
T = 128
NB = N // T  # 32
P = 128
NCORES = 8

INPUT_DTYPE = "float16"
OUT_DTYPE = "float16"
PRELOAD_OUTSIDE = False  # timing ablation: input DMAs outside repeat loop
OUT_ENGINE = "gpsimd"    # which queue issues output DMAs
PSUM_BUFS = 8  # PSUM allocation is bank-granular: 8 banks max
NPAN = 64
NTRI = 136
NEX = 136
NSLOTS = 126
CH = 8  # tiles per DMA chunk (input stacks and output staging)
NSLOTS_PAD = 128  # 16 chunks of 8
O_BUFS = 3  # out-staging chunks in flight


def trii16(r, c):
    """triu-16 row-major index, 0 <= r <= c < 16."""
    assert 0 <= r <= c < 16
    return r * 16 - r * (r - 1) // 2 + (c - r)


# ---------------- template ----------------
# ex layout (per-core data, fixed reference pattern):
E_CL8S = 0      # 2x8  col-L8 stats
E_RL8S = 16     # 2x8  row-L8 stats (reversed)
E_RL8M = 32     # 36   row-L8 movs (shared by both row-L8 ladders)
E_CL4S0 = 68    # 4    col-L4 ss0 stats
E_CL4S1 = 72    # 4    col-L4 ss1 stats
E_RL4S0 = 76    # 4    row-L4 ss0 stats (reversed)
E_RL4M0 = 80    # 10   row-L4 ss0 movs
E_RL4S1 = 90    # 4    row-L4 ss1 stats (reversed)
E_RL4M1 = 94    # 10   row-L4 ss1 movs
E_CL2 = 104     # 2x(2+3) col-L2 a/b stats+movs
E_RL2 = 114     # 2x(2+3) row-L2 a/b stats+movs
E_S2 = 124      # 2x6  s2 cleanup
assert E_S2 + 12 == NEX


def tri8(r, c):
    """triu-8 row-major index, r <= c < 8 (used for row-L8 mov layout)."""
    return r * 8 - r * (r - 1) // 2 + (c - r)


def tri4(r, c):
    return r * 4 - r * (r - 1) // 2 + (c - r)


def _build_template():
    chains = []
    slot = [0]

    def add(units):
        chains.append(dict(units=units, out=slot[0]))
        slot[0] += 1

    # 4 col-L16 ladders
    for l in range(4):
        for c in range(16):
            add([(("pan", l * 16 + c + u), ("tri", trii16(c, c + u)))
                 for u in range(16 - c)])
    # 2 col-L8 (movs: tri triu[0,8) corner)
    for l in range(2):
        for c in range(8):
            add([(("ex", E_CL8S + l * 8 + c + u), ("tri", trii16(c, c + u)))
                 for u in range(8 - c)])
    # 2 row-L8 (stats ex reversed; movs shared ex triu-8 block)
    for l in range(2):
        for c in range(8):
            add([(("ex", E_RL8S + l * 8 + c + u),
                  ("ex", E_RL8M + tri8(7 - c - u, 7 - c)))
                 for u in range(8 - c)])
    # col-L4 ss0 (movs tri triu[0,4)), ss1 (movs tri triu[8,12))
    for c in range(4):
        add([(("ex", E_CL4S0 + c + u), ("tri", trii16(c, c + u)))
             for u in range(4 - c)])
    for c in range(4):
        add([(("ex", E_CL4S1 + c + u), ("tri", trii16(8 + c, 8 + c + u)))
             for u in range(4 - c)])
    # row-L4 ss0 / ss1 (stats reversed, movs ex)
    for base_s, base_m in ((E_RL4S0, E_RL4M0), (E_RL4S1, E_RL4M1)):
        for c in range(4):
            add([(("ex", base_s + c + u),
                  ("ex", base_m + tri4(3 - c - u, 3 - c)))
                 for u in range(4 - c)])
    # col-L2 a/b: stats ex[s..s+1], movs ex[s+2..s+4]
    for g in range(2):
        s = E_CL2 + g * 5
        add([(("ex", s + 0), ("ex", s + 2)), (("ex", s + 1), ("ex", s + 3))])
        add([(("ex", s + 1), ("ex", s + 4))])
    # row-L2 a/b
    for g in range(2):
        s = E_RL2 + g * 5
        add([(("ex", s + 0), ("ex", s + 2)), (("ex", s + 1), ("ex", s + 3))])
        add([(("ex", s + 1), ("ex", s + 4))])
    # s2 x2: merged C(b,b+1) len-2 chain + 2 singles
    for g in range(2):
        s = E_S2 + g * 6
        add([(("ex", s + 0), ("ex", s + 3)), (("ex", s + 1), ("ex", s + 4))])
        add([(("ex", s + 0), ("ex", s + 5))])
        add([(("ex", s + 2), ("ex", s + 4))])
    assert slot[0] == NSLOTS, slot[0]
    assert sum(len(ch["units"]) for ch in chains) == 748
    return chains


TEMPLATE = _build_template()


def _acore_filling(c):
    pan, tri, ex = {}, {}, {}
    outs = [None] * NSLOTS

    # 4 col-L16 ladders: J = 16+4c..16+4c+3
    Jl = [16 + 4 * c + l for l in range(4)]
    for l in range(4):
        for t in range(16):
            pan[l * 16 + t] = ("B", t, Jl[l])
    for r in range(16):
        for s in range(r, 16):
            tri[trii16(r, s)] = ("A", r, s)
    # col-L8: J8 in {8+c, 12+c}
    J8 = [8 + c, 12 + c]
    for l in range(2):
        for t in range(8):
            ex[E_CL8S + l * 8 + t] = ("B", t, J8[l])
    # row-L8: I8 in {2c, 2c+1}; stats reversed A(I8, 15-t); movs B-triu[8,16)
    I8 = [2 * c, 2 * c + 1]
    for l in range(2):
        for t in range(8):
            ex[E_RL8S + l * 8 + t] = ("A", I8[l], 15 - t)
    for r in range(8):
        for s in range(r, 8):
            ex[E_RL8M + tri8(r, s)] = ("B", 8 + r, 8 + s)
    # col-L4: ss0 J=4+c stats B(t,J); ss1 J=12+c stats B(8+t, J)
    J4a, J4b = 4 + c, 12 + c
    for t in range(4):
        ex[E_CL4S0 + t] = ("B", t, J4a)
        ex[E_CL4S1 + t] = ("B", 8 + t, J4b)
    # row-L4: ss0 I=c stats A(c, 7-t), movs B-triu[4,8); ss1 I=8+c
    I4a, I4b = c, 8 + c
    for t in range(4):
        ex[E_RL4S0 + t] = ("A", I4a, 7 - t)
        ex[E_RL4S1 + t] = ("A", I4b, 15 - t)
    for r in range(4):
        for s in range(r, 4):
            ex[E_RL4M0 + tri4(r, s)] = ("B", 4 + r, 4 + s)
            ex[E_RL4M1 + tri4(r, s)] = ("B", 12 + r, 12 + s)
    # col-L2 a/b: (b, J) pairs
    cl2 = [(4 * c, 4 * c + 2), (4 * ((c + 1) % 4), 4 * ((c + 1) % 4) + 3)]
    for g, (b, J) in enumerate(cl2):
        s = E_CL2 + g * 5
        ex[s + 0] = ("B", b, J)
        ex[s + 1] = ("B", b + 1, J)
        ex[s + 2] = ("A", b, b)
        ex[s + 3] = ("A", b, b + 1)
        ex[s + 4] = ("A", b + 1, b + 1)
    # row-L2 a/b: (b, I) pairs
    rl2 = [(4 * ((c + 2) % 4), 4 * ((c + 2) % 4)),
           (4 * ((c + 3) % 4), 4 * ((c + 3) % 4) + 1)]
    for g, (b, I) in enumerate(rl2):
        s = E_RL2 + g * 5
        ex[s + 0] = ("A", I, b + 3)
        ex[s + 1] = ("A", I, b + 2)
        ex[s + 2] = ("B", b + 3, b + 3)
        ex[s + 3] = ("B", b + 2, b + 3)
        ex[s + 4] = ("B", b + 2, b + 2)
    # s2 x2 at b2 in {4c, 4c+2}
    b2s = [4 * c, 4 * c + 2]
    for g, b in enumerate(b2s):
        s = E_S2 + g * 6
        ex[s + 0] = ("A", b, b)
        ex[s + 1] = ("A", b, b + 1)
        ex[s + 2] = ("A", b + 1, b + 1)
        ex[s + 3] = ("B", b, b + 1)
        ex[s + 4] = ("B", b + 1, b + 1)
        ex[s + 5] = ("B", b, b)

    # ---- out slots in template order ----
    slot = 0
    for l in range(4):
        for cc in range(16):
            outs[slot] = (cc, Jl[l], True); slot += 1
    for l in range(2):
        for cc in range(8):
            outs[slot] = (cc, J8[l], True); slot += 1
    for l in range(2):
        for cc in range(8):
            outs[slot] = (I8[l], 15 - cc, False); slot += 1
    for cc in range(4):
        outs[slot] = (cc, J4a, True); slot += 1
    for cc in range(4):
        outs[slot] = (8 + cc, J4b, True); slot += 1
    for cc in range(4):
        outs[slot] = (I4a, 7 - cc, False); slot += 1
    for cc in range(4):
        outs[slot] = (I4b, 15 - cc, False); slot += 1
    for g, (b, J) in enumerate(cl2):
        outs[slot] = (b, J, True); slot += 1
        outs[slot] = (b + 1, J, True); slot += 1
    for g, (b, I) in enumerate(rl2):
        outs[slot] = (I, b + 3, False); slot += 1
        outs[slot] = (I, b + 2, False); slot += 1
    for g, b in enumerate(b2s):
        outs[slot] = (b, b + 1, False); slot += 1
        outs[slot] = (b, b, False); slot += 1
        outs[slot] = (b + 1, b + 1, False); slot += 1
    assert slot == NSLOTS
    return dict(pan=pan, tri=tri, ex=ex), outs


def _bcore_filling(c):
    fill, outs = _acore_filling(c - 4)
    tfill = {}
    for stack, mp in fill.items():
        tfill[stack] = {}
        for idx, (mat, bi, bj) in mp.items():
            tfill[stack][idx] = (("B", 31 - bj, 31 - bi) if mat == "A"
                                 else ("A", 31 - bj, 31 - bi))
    touts = [(31 - J, 31 - I, not tr) for (I, J, tr) in outs]
    return tfill, touts


_FILLINGS = [(_acore_filling(c) if c < 4 else _bcore_filling(c))
             for c in range(NCORES)]


def _check_cover():
    seen = {}
    for c in range(NCORES):
        fill, outs = _FILLINGS[c]
        for ch in TEMPLATE:
            I, J, transposed = outs[ch["out"]]
            for (ss, si), (ms_, mi) in ch["units"]:
                sb = fill[ss][si]
                mb = fill[ms_][mi]
                ab = sb if sb[0] == "A" else mb
                bb = sb if sb[0] == "B" else mb
                assert ab[0] == "A" and bb[0] == "B", (c, sb, mb)
                assert ab[1] == I and bb[2] == J, (c, I, J, ab, bb)
                K = ab[2]
                assert bb[1] == K, (c, I, J, K, ab, bb)
                assert I <= K <= J, (c, I, K, J)
                assert transposed == (sb[0] == "B"), (c, ch["out"], sb)
                key = (I, K, J)
                assert key not in seen, (key, seen.get(key), c)
                seen[key] = c
    want = {(i, k, j) for i in range(NB) for k in range(i, NB)
            for j in range(k, NB)}
    assert set(seen) == want, (len(seen), len(want))


_check_cover()

_PROGRAMS = {}


def _build_program(repeat=1):
    import contextlib
    import concourse.bacc as bacc
    import concourse.mybir as mybir
    from concourse.tile import TileContext

    dt_in = getattr(mybir.dt, INPUT_DTYPE)
    dt_out = getattr(mybir.dt, OUT_DTYPE)
    f32 = mybir.dt.float32
    nc = bacc.Bacc("TRN2", target_bir_lowering=False, debug=False,
                   num_devices=NCORES)
    # chunked stacks: [nchunks, P, CH, T] so one dma_start moves CH tiles
    pan_in = nc.dram_tensor("pan", [NPAN // CH, P, CH, T], dt_in,
                            kind="ExternalInput")
    tri_in = nc.dram_tensor("tri", [NTRI // CH, P, CH, T], dt_in,
                            kind="ExternalInput")
    ex_in = nc.dram_tensor("ex", [NEX // CH, P, CH, T], dt_in,
                           kind="ExternalInput")
    c_out = nc.dram_tensor("out_stack", [NSLOTS_PAD // CH, P, CH, T], dt_out,
                           kind="ExternalOutput")

    with TileContext(nc) as tc:
        with (
            # 2x chunks: double-buffer across repeat iterations so next-iter
            # input DMAs land while this iteration computes
            tc.tile_pool(name="pan_pool", bufs=2 * NPAN // CH) as pan_pool,
            tc.tile_pool(name="tri_pool", bufs=2 * NTRI // CH) as tri_pool,
            tc.tile_pool(name="ex_pool", bufs=2 * NEX // CH) as ex_pool,
            tc.tile_pool(name="o_pool", bufs=O_BUFS) as o_pool,
            tc.tile_pool(name="psum", bufs=PSUM_BUFS, space="PSUM") as psum_pool,
        ):
            loop_ctx = (tc.For_i(0, repeat, 1) if repeat > 1
                        else contextlib.nullcontext())
            with loop_ctx:
                chunks = {}

                def load(stack, pool, cidx):
                    t_ = pool.tile([P, CH, T], dt_in, tag=stack,
                                   name=f"{stack}_{cidx}")
                    src = {"pan": pan_in, "tri": tri_in, "ex": ex_in}[stack]
                    nc.sync.dma_start(out=t_, in_=src[cidx])
                    chunks[(stack, cidx)] = t_

                for t in range(2):
                    load("pan", pan_pool, t)
                for i in range(NTRI // CH):
                    load("tri", tri_pool, i)
                for t in range(2, NPAN // CH):
                    load("pan", pan_pool, t)
                for i in range(NEX // CH):
                    load("ex", ex_pool, i)

                def ap(ref):
                    stack, idx = ref
                    return chunks[(stack, idx // CH)][:, idx % CH, :]

                o_t = None
                for ci, ch in enumerate(TEMPLATE):
                    units = ch["units"]
                    L = len(units)
                    ps = psum_pool.tile([P, T], f32, tag="ps",
                                        name=f"ps_{ch['out']}")
                    for u, (sref, mref) in enumerate(units):
                        nc.tensor.matmul(
                            ps[:, :], ap(sref), ap(mref),
                            start=(u == 0), stop=(u == L - 1),
                        )
                    if ci % CH == 0:
                        o_t = o_pool.tile([P, CH, T], dt_out, tag="o",
                                          name=f"o_{ci // CH}")
                    if ci % 2 == 0:
                        nc.vector.tensor_copy(o_t[:, ci % CH, :], ps[:, :])
                    else:
                        nc.scalar.copy(o_t[:, ci % CH, :], ps[:, :])
                    if ci % CH == CH - 1:
                        nc.gpsimd.dma_start(out=c_out[ci // CH], in_=o_t)
                    elif ci == NSLOTS - 1:  # partial last chunk: written lanes only
                        nlanes = NSLOTS - (ci // CH) * CH
                        nc.gpsimd.dma_start(out=c_out[ci // CH][:, :nlanes, :],
                                            in_=o_t[:, :nlanes, :])
    nc.finalize()
    return nc


def _get_program(repeat=1):
    if repeat not in _PROGRAMS:
        _PROGRAMS[repeat] = _build_program(repeat)
    return _PROGRAMS[repeat]


def _build_in_maps(A, B):
    tri_mask = np.triu(np.ones((T, T), dtype=np.float32))
    np_in = np.float16 if INPUT_DTYPE == "float16" else np.float32
    cache = {}

    def get_block(mat, bi, bj):
        key = (mat, bi, bj)
        if key not in cache:
            M = A if mat == "A" else B
            blk = M[bi * T:(bi + 1) * T, bj * T:(bj + 1) * T]
            if bi == bj:
                blk = blk * tri_mask
            packed = np.ascontiguousarray(blk.T) if mat == "A" else blk
            cache[key] = packed.astype(np_in)
        return cache[key]

    in_maps = []
    for c in range(NCORES):
        fill, _ = _FILLINGS[c]
        m = {}
        for stack, size in [("pan", NPAN), ("tri", NTRI), ("ex", NEX)]:
            arr = np.empty((size, P, T), dtype=np_in)
            for idx in range(size):
                mat, bi, bj = fill[stack][idx]
                arr[idx] = get_block(mat, bi, bj)
            # [size,P,T] -> chunked [size//CH, P, CH, T]
            m[stack] = np.ascontiguousarray(
                arr.reshape(size // CH, CH, P, T).transpose(0, 2, 1, 3))
        in_maps.append(m)
    return in_maps


def _unpack(results):
    C = np.zeros((N, N), dtype=np.float32)
    for c in range(NCORES):
        out = results[c]["out_stack"].astype(np.float32)
        # [nchunks,P,CH,T] -> [NSLOTS_PAD,P,T]
        out = out.transpose(0, 2, 1, 3).reshape(NSLOTS_PAD, P, T)
        _, outs = _FILLINGS[c]
        for s, (oi, oj, transposed) in enumerate(outs):
            part = out[s]
            if transposed:
                part = part.T
            C[oi * T:(oi + 1) * T, oj * T:(oj + 1) * T] += part
    return C


def _emulate(A, B):
    in_maps = _build_in_maps(A, B)
    results = []
    for c in range(NCORES):
        m = in_maps[c]

        def tile(ref):
            stack, idx = ref
            return m[stack][idx // CH][:, idx % CH, :].astype(np.float32)

        out = np.zeros((NSLOTS_PAD, P, T), dtype=np.float32)
        for ch in TEMPLATE:
            ps = np.zeros((P, T), dtype=np.float32)
            for sref, mref in ch["units"]:
                ps += tile(sref).T @ tile(mref)
            out[ch["out"]] = ps
        results.append({"out_stack": np.ascontiguousarray(
            out.reshape(NSLOTS_PAD // CH, CH, P, T).transpose(0, 2, 1, 3))})
    return _unpack(results)


def kernel(A, B):
    from concourse.bass_utils import run_bass_kernel_spmd

    A = np.asarray(A, dtype=np.float32)
    B = np.asarray(B, dtype=np.float32)
    nc = _get_program()
    in_maps = _build_in_maps(A, B)
    res = run_bass_kernel_spmd(nc, in_maps, list(range(NCORES)))
    return _unpack(res.results)


if __name__ == "__main__":
    rng = np.random.default_rng(0)
    A = rng.standard_normal((N, N), dtype=np.float32)
    B = rng.standard_normal((N, N), dtype=np.float32)
    ref = np.triu(np.triu(A).astype(np.float32) @ np.triu(B))
    got = _emulate(A, B)
    rel = np.linalg.norm(got - ref) / np.linalg.norm(ref)
    print(f"emulation rel err: {rel:.3e}")
    assert rel < 2e-3, rel
    print("emulation OK")

